# revision 6
# baseline (speedup 1.0000x reference)
"""DeepPoly ReLU relaxation (elementwise) on 8 TRN2 NeuronCores.

Full inputs x, low, high: (8, 2048, 2048) f32. Output tuple:
  x_out    = relu(x)
  low_out  = low * [(high + min(low, 0)) > 0]
  high_out = relu(high)

(low_out mask == reference's crossing/lam logic; verified bit-exact on the
fixed inputs. high_out's ub_slope*high + ub_int algebraically equals high.)

Sharding: batch element i -> core i. No communication.
"""

import numpy as np

import concourse.bacc as bacc
import concourse.mybir as mybir
from concourse.tile import TileContext
from concourse.bass_utils import run_bass_kernel_spmd

N_CORES = 8
# Per-core slice is 2048*2048 f32, viewed as (1024, 4096) for [128, 4096] tiles.
RV, FV = 1024, 4096
P = 128
N_TILES = RV // P  # 8
F32 = mybir.dt.float32
Op = mybir.AluOpType
Act = mybir.ActivationFunctionType


def build_nc(reps=1, fv=FV, bufs=3, out_engine="sync"):
    nc = bacc.Bacc("TRN2", target_bir_lowering=False, debug=False)

    rv = (RV * FV) // fv
    n_tiles = rv // P
    x = nc.dram_tensor("x", [rv, fv], F32, kind="ExternalInput")
    low = nc.dram_tensor("low", [rv, fv], F32, kind="ExternalInput")
    high = nc.dram_tensor("high", [rv, fv], F32, kind="ExternalInput")
    x_out = nc.dram_tensor("x_out", [rv, fv], F32, kind="ExternalOutput")
    low_out = nc.dram_tensor("low_out", [rv, fv], F32, kind="ExternalOutput")
    high_out = nc.dram_tensor("high_out", [rv, fv], F32, kind="ExternalOutput")

    with TileContext(nc) as tc:
        with tc.tile_pool(name="sbuf", bufs=bufs) as pool:
            for i in range(n_tiles * reps):
                i = i % n_tiles
                rows = slice(i * P, (i + 1) * P)
                tx = pool.tile([P, fv], F32, tag="tx")
                tl = pool.tile([P, fv], F32, tag="tl")
                th = pool.tile([P, fv], F32, tag="th")
                ts = pool.tile([P, fv], F32, tag="ts")

                nc.sync.dma_start(out=tx[:], in_=x[rows])
                nc.sync.dma_start(out=tl[:], in_=low[rows])
                nc.sync.dma_start(out=th[:], in_=high[rows])

                # s = min(low, 0) + high
                nc.vector.scalar_tensor_tensor(
                    out=ts[:], in0=tl[:], scalar=0.0, in1=th[:],
                    op0=Op.min, op1=Op.add,
                )
                # low_out = (s > 0) * low
                nc.vector.scalar_tensor_tensor(
                    out=ts[:], in0=ts[:], scalar=0.0, in1=tl[:],
                    op0=Op.is_gt, op1=Op.mult,
                )
                # relus in place (ordered after the reads above by Tile deps)
                nc.scalar.activation(out=th[:], in_=th[:], func=Act.Relu)
                nc.scalar.activation(out=tx[:], in_=tx[:], func=Act.Relu)

                oe = getattr(nc, out_engine)
                oe.dma_start(out=x_out[rows], in_=tx[:])
                oe.dma_start(out=low_out[rows], in_=ts[:])
                oe.dma_start(out=high_out[rows], in_=th[:])

    # Under axon, run_bass_kernel_spmd skips the native path's finalize.
    nc.finalize()
    return nc


def build_nc_dmaonly(reps=1, fv=FV, bufs=3, out_engine="sync"):
    """Pure DMA round-trip with no compute: measures the HBM roofline."""
    nc = bacc.Bacc("TRN2", target_bir_lowering=False, debug=False)
    rv = (RV * FV) // fv
    n_tiles = rv // P
    ios = []
    for name in ("x", "low", "high"):
        t_in = nc.dram_tensor(name, [rv, fv], F32, kind="ExternalInput")
        t_out = nc.dram_tensor(name + "_out", [rv, fv], F32, kind="ExternalOutput")
        ios.append((t_in, t_out))
    with TileContext(nc) as tc:
        with tc.tile_pool(name="sbuf", bufs=bufs) as pool:
            for i in range(n_tiles * reps):
                i = i % n_tiles
                rows = slice(i * P, (i + 1) * P)
                for name, (t_in, t_out) in zip(("tx", "tl", "th"), ios):
                    t = pool.tile([P, fv], F32, tag=name)
                    nc.sync.dma_start(out=t[:], in_=t_in[rows])
                    getattr(nc, out_engine).dma_start(out=t_out[rows], in_=t[:])
    nc.finalize()
    return nc


def run(x, low, high, trace=False, **spmd_kwargs):
    """x, low, high: (8, 2048, 2048) f32 numpy. Returns (outputs_tuple, BassKernelResults)."""
    nc = build_nc()
    in_maps = [
        {
            "x": np.ascontiguousarray(x[i]).reshape(RV, FV),
            "low": np.ascontiguousarray(low[i]).reshape(RV, FV),
            "high": np.ascontiguousarray(high[i]).reshape(RV, FV),
        }
        for i in range(N_CORES)
    ]
    br = run_bass_kernel_spmd(nc, in_maps, list(range(N_CORES)), trace=trace, **spmd_kwargs)
    res = br.results
    outs = tuple(
        np.stack([np.asarray(res[i][name]).reshape(2048, 2048) for i in range(N_CORES)])
        for name in ("x_out", "low_out", "high_out")
    )
    return outs, br


def kernel(x, low, high):
    x = np.asarray(x, dtype=np.float32)
    low = np.asarray(low, dtype=np.float32)
    high = np.asarray(high, dtype=np.float32)
    outs, _ = run(x, low, high, trace=False)
    return outs
